# revision 26
# baseline (speedup 1.0000x reference)
"""CT self-attention (causal + 2 future frames) for Trainium2, 8 NeuronCores.

Sharding: batch (4-way) x head-group (2-way): core c = 2*b + g handles batch b,
heads [8g, 8g+8). Each core computes its QKV projection slice, banded
attention for its 8 heads, and a partial output projection; the host sums the
two partial outputs per batch and adds the (host-folded) biases.

The kernel is Scalar-engine bound (the banded softmax exp is ~116us of
ScalarE work per core), so everything is organized to keep the exp pipeline
fed from ~15us onward:
  - pair-major attention: pair p sweeps all 4 query blocks (43 key-tiles of
    exp work) while the NEXT pair's Q/K projection chunks are dribbled in
    every 5th tile, and V-projection tiles are dribbled into pair 0
  - scores: S_T = K_h^T-tile.T @ Q_h, 2 heads packed in the 128-row PE array
    via tile_position row tiling (the two instructions stream concurrently)
  - CT mask: gpsimd multiplies the exp output by a 0/1 keep-mask on diagonal
    tiles (no PE mask matmuls)
  - softmax: exp on ScalarE with scale=1/8 and per-key padding bias; no max
    subtraction (|s|/8 <= ~6 for N(0,1) inputs); the denominator comes free
    from a ones column appended to V (M=65 AV matmul -> partition 64)
  - AV: attnT = V-tile.T @ E accumulated over key tiles
  - normalize + output projection are deferred ops drained one per key-tile
    slot (kt>=4) of later blocks, so the PE never bursts long enough to
    starve ScalarE
All matmuls bfloat16 (1 row/cycle, half the DMA of f32).
"""
import math
from collections import deque
from contextlib import ExitStack

import numpy as np

B, T, D, H = 4, 2048, 1024, 16
HD = D // H            # 64
L = 2                  # max_future_frames
NCORES = 8
HPG = 8                # heads per group/core
NPAIR = 4              # head pairs per core
FCH = 8                # feature chunks (D / 128)
TQ5 = 4                # 512-wide query tiles
NKT = 16               # 128-wide key tiles
NEG = -1.0e9

_BUILT = {}


def _build_nc():
    import concourse.tile as tile
    from concourse import bacc, mybir

    dt = mybir.dt
    f32, f32r, bf16 = dt.float32, dt.float32r, dt.bfloat16
    Exp = mybir.ActivationFunctionType.Exp
    MUL = mybir.AluOpType.mult
    ADD = mybir.AluOpType.add

    nc = bacc.Bacc(None, target_bir_lowering=False)
    xT_d = nc.dram_tensor("xT", [FCH, TQ5, 128, 512], bf16, kind="ExternalInput")
    wqk_d = nc.dram_tensor("wqk", [8, FCH, 128, 128], bf16, kind="ExternalInput")
    wv_d = nc.dram_tensor("wv", [FCH, 128, 512], bf16, kind="ExternalInput")
    woutT_d = nc.dram_tensor("woutT", [NPAIR, 128, D], bf16, kind="ExternalInput")
    bq_d = nc.dram_tensor("bq", [128, NPAIR], f32, kind="ExternalInput")
    bk_d = nc.dram_tensor("bk", [128, NPAIR], f32, kind="ExternalInput")
    kpb_d = nc.dram_tensor("kpb", [128, NKT], f32, kind="ExternalInput")
    m01_d = nc.dram_tensor("m01", [128, 5, 512], bf16, kind="ExternalInput")
    selbc_d = nc.dram_tensor("selbc", [2, 2 * HD], f32r, kind="ExternalInput")
    vones_d = nc.dram_tensor("vones", [128, NKT * HPG], bf16, kind="ExternalInput")
    out_d = nc.dram_tensor("out_part", [T, D], f32, kind="ExternalOutput")

    with tile.TileContext(nc) as tc, \
         nc.allow_low_precision(reason="bf16 matmul fast path"), \
         ExitStack() as top:
        pers = top.enter_context(tc.tile_pool(name="pers", bufs=1))
        QT = pers.tile([128, NPAIR, T], bf16, name="QT")
        KT = pers.tile([128, NPAIR, T], bf16, name="KT")
        Vt = pers.tile([128, NKT, HPG, HD + 1], bf16, name="Vt")
        m01_sb = pers.tile([128, 5, 512], bf16, name="m01_sb")
        kp_sb = pers.tile([128, NKT], f32, name="kp_sb")
        bq_sb = pers.tile([128, NPAIR], f32, name="bq_sb")
        bk_sb = pers.tile([128, NPAIR], f32, name="bk_sb")
        selbc_sb = pers.tile([2, 2 * HD], f32r, name="selbc_sb")
        vones_sb = pers.tile([128, NKT * HPG], bf16, name="vones_sb")
        nc.gpsimd.dma_start(m01_sb[:], m01_d[:])
        nc.gpsimd.dma_start(kp_sb[:], kpb_d[:])
        nc.gpsimd.dma_start(bq_sb[:], bq_d[:])
        nc.gpsimd.dma_start(bk_sb[:], bk_d[:])
        nc.gpsimd.dma_start(selbc_sb[:], selbc_d[:])
        nc.gpsimd.dma_start(vones_sb[:], vones_d[:])
        nc.vector.tensor_copy(
            Vt[:, :, :, HD],
            vones_sb[:].rearrange("p (a b) -> p a b", a=NKT))

        # x chunks land t5-major so the first Q/K chunks start ~3us in;
        # weight DMAs for pair 0 (tgt 0 and 4) are queued first.
        xsp = top.enter_context(tc.tile_pool(name="xs", bufs=1))
        wqp = top.enter_context(tc.tile_pool(name="wq", bufs=1))
        xT_sb = xsp.tile([128, FCH, T], bf16, name="xT_sb")
        for t5 in range(TQ5):
            for f in range(FCH):
                nc.gpsimd.dma_start(
                    xT_sb[:, f, t5 * 512:(t5 + 1) * 512], xT_d[f, t5])
        wqk_sb = wqp.tile([128, 8, FCH, 128], bf16, name="wqk_sb")
        wv_sb = pers.tile([128, FCH, 512], bf16, name="wv_sb")
        for tgt in (0, 4):
            for f in range(FCH):
                nc.sync.dma_start(wqk_sb[:, tgt, f], wqk_d[tgt, f])
        for f in range(FCH):
            nc.sync.dma_start(wv_sb[:, f, :], wv_d[f])
        for tgt in (1, 5, 2, 6, 3, 7):
            for f in range(FCH):
                nc.sync.dma_start(wqk_sb[:, tgt, f], wqk_d[tgt, f])

        with tc.tile_pool(name="pers2", bufs=1) as pers2, \
             tc.tile_pool(name="eps", bufs=4) as epool, \
             tc.tile_pool(name="nsb", bufs=3) as nsb, \
             tc.tile_pool(name="avp", bufs=2) as avp, \
             tc.tile_pool(name="wo", bufs=1) as wop, \
             tc.tile_pool(name="osb", bufs=2) as osb, \
             tc.tile_pool(name="psAv", bufs=1, space="PSUM") as psAv, \
             tc.tile_pool(name="psSc", bufs=2, space="PSUM") as psSc, \
             tc.tile_pool(name="psC", bufs=1, space="PSUM") as psC:
            AT = pers2.tile([128, NPAIR, T], bf16, name="AT")
            wo_sb = wop.tile([128, NPAIR, D], bf16, name="wo_sb")
            for cchunk in range(NPAIR):
                nc.gpsimd.dma_start(wo_sb[:, cchunk, :], woutT_d[cchunk])

            def emit_qk_chunk(tgt, t5):
                # one [128 out-rows, 512 queries] Q/K projection chunk
                pair = tgt % 4
                pq = psC.tile([128, 512], f32, name="pqk", tag=f"po{t5 % 2}")
                for f in range(FCH):
                    nc.tensor.matmul(
                        pq[:], wqk_sb[:, tgt, f],
                        xT_sb[:, f, t5 * 512:(t5 + 1) * 512],
                        start=(f == 0), stop=(f == FCH - 1))
                dst = (QT if tgt < 4 else KT)[:, pair, t5 * 512:(t5 + 1) * 512]
                bias = (bq_sb if tgt < 4 else bk_sb)[:, pair:pair + 1]
                nc.vector.tensor_scalar(dst, pq[:], bias, None, ADD)

            def emit_v_tile(t):
                pv = psC.tile([128, 512], f32, name="pv", tag=f"po{t % 2}")
                for f in range(FCH):
                    nc.tensor.matmul(
                        pv[:], xT_sb[:, f, t * 128:(t + 1) * 128],
                        wv_sb[:, f, :],
                        start=(f == 0), stop=(f == FCH - 1))
                nc.vector.tensor_copy(
                    Vt[:, t, :, 0:HD],
                    pv[:].rearrange("p (h d) -> p h d", h=HPG))

            def mk_norm(p, q5, hh, avs, recp):
                def run():
                    qs = slice(q5 * 512, (q5 + 1) * 512)
                    bc = psSc.tile([64, 512], f32, name="bc", tag="sc2")
                    nc.tensor.matmul(
                        bc[:], selbc_sb[:, hh * HD:(hh + 1) * HD], recp[:],
                        start=True, stop=True)
                    nc.vector.tensor_tensor(
                        AT[64 * hh:64 * (hh + 1), p, qs],
                        avs[0:64, :], bc[:], MUL)
                return run

            def mk_proj(q5, tq, wide=False):
                def run():
                    t = 4 * q5 + tq
                    tsl = slice(t * 128, (t + 1) * 128)
                    if wide and tq % 2 == 1:
                        po0 = psSc.tile([128, 512], f32, name="po0w", tag="sc2")
                        po1 = psSc.tile([128, 512], f32, name="po1w", tag="sc2")
                    else:
                        po0 = psC.tile([128, 512], f32, name="po0", tag="po0")
                        po1 = psC.tile([128, 512], f32, name="po1", tag="po1")
                    for cchunk in range(NPAIR):
                        lhsT = AT[:, cchunk, tsl]
                        nc.tensor.matmul(po0[:], lhsT, wo_sb[:, cchunk, 0:512],
                                         start=(cchunk == 0), stop=(cchunk == 3))
                        nc.tensor.matmul(po1[:], lhsT, wo_sb[:, cchunk, 512:1024],
                                         start=(cchunk == 0), stop=(cchunk == 3))
                    ot = osb.tile([128, D], f32, name="ot", tag="ot")
                    nc.vector.tensor_copy(ot[:, 0:512], po0[:])
                    nc.vector.tensor_copy(ot[:, 512:1024], po1[:])
                    nc.sync.dma_start(out_d[tsl, :], ot[:])
                return run

            # warm-up: pair 0's Q/K for the first query block only (the
            # rest dribbles into pair 0's tile slots), then 5 V tiles
            emit_qk_chunk(4, 0)
            emit_qk_chunk(0, 0)
            for t in range(5):
                emit_v_tile(t)

            deferred = deque()
            for p in range(NPAIR):
                # work dribbled into this pair's tile slots:
                qkv_chunks = deque()
                if p == 0:
                    for t5 in range(1, TQ5):
                        qkv_chunks.append((4, t5))
                        qkv_chunks.append((0, t5))
                if p < 3:
                    for t5 in range(TQ5):
                        qkv_chunks.append((p + 1, t5))
                        qkv_chunks.append((5 + p, t5))
                v_tiles = deque(range(5, NKT)) if p == 0 else deque()
                slot = 0
                for q5 in range(TQ5):
                    nkt = min(4 * q5 + 5, NKT)
                    q5s = q5 * 512
                    avA = psAv.tile([HD + 1, 512], f32, name="avA", tag="avA")
                    avB = psAv.tile([HD + 1, 512], f32, name="avB", tag="avB")
                    pend_av = None
                    for kt in range(nkt):
                        ks = slice(kt * 128, (kt + 1) * 128)
                        off = kt - 4 * q5
                        masked = off >= 0
                        q0 = max(0, 128 * off - L) if masked else 0
                        qs = slice(q5s + q0, q5s + 512)
                        sc2 = psSc.tile([128, 2, 512], f32, name="sc2", tag="sc2")
                        nc.tensor.matmul(sc2[:, 0, q0:512],
                                         KT[0:64, p, ks], QT[0:64, p, qs],
                                         start=True, stop=True,
                                         tile_position=(0, 0))
                        nc.tensor.matmul(sc2[:, 1, q0:512],
                                         KT[64:128, p, ks], QT[64:128, p, qs],
                                         start=True, stop=True,
                                         tile_position=(64, 0))
                        if pend_av is not None:
                            pend_av()
                            pend_av = None
                        e2 = epool.tile([128, 2, 512], bf16, name="e2", tag="e2")
                        nc.scalar.activation(e2[:, :, q0:512], sc2[:, :, q0:512],
                                             Exp, bias=kp_sb[:, kt:kt + 1],
                                             scale=1.0 / math.sqrt(HD))
                        if masked:
                            m1 = min(512, 128 * off + 126)
                            nc.gpsimd.tensor_tensor(
                                e2[:, 0, q0:m1], e2[:, 0, q0:m1],
                                m01_sb[:, off, q0:m1], MUL)
                            nc.gpsimd.tensor_tensor(
                                e2[:, 1, q0:m1], e2[:, 1, q0:m1],
                                m01_sb[:, off, q0:m1], MUL)

                        def mk_av(kt=kt, e2=e2, q0=q0, avA=avA, avB=avB,
                                  p=p, nkt=nkt):
                            nc.tensor.matmul(avA[0:65, q0:512],
                                             Vt[:, kt, 2 * p, :],
                                             e2[:, 0, q0:512],
                                             start=(kt == 0), stop=(kt == nkt - 1),
                                             skip_group_check=True)
                            nc.tensor.matmul(avB[0:65, q0:512],
                                             Vt[:, kt, 2 * p + 1, :],
                                             e2[:, 1, q0:512],
                                             start=(kt == 0), stop=(kt == nkt - 1),
                                             skip_group_check=True)
                        pend_av = mk_av

                        # dribble deferred work into this slot
                        if v_tiles and slot >= 1:
                            emit_v_tile(v_tiles.popleft())
                            if v_tiles and slot % 2 == 0:
                                emit_v_tile(v_tiles.popleft())
                        if qkv_chunks and (
                                slot % 3 == 2
                                or (len(qkv_chunks) > 8 and slot % 2 == 0)):
                            tgt, t5 = qkv_chunks.popleft()
                            emit_qk_chunk(tgt, t5)
                        elif deferred and kt >= 4:
                            deferred.popleft()()
                        slot += 1
                    pend_av()
                    # evacuate AV (frees banks) + gather the denominators
                    dpool = nsb.tile([2, 512], f32, name="dpool", tag="dpool")
                    avs2 = []
                    for hh, av in ((0, avA), (1, avB)):
                        avs = avp.tile([HD + 1, 512], f32,
                                       name=f"avs{hh}", tag=f"avs{hh}")
                        nc.vector.tensor_copy(avs[:], av[:])
                        nc.sync.dma_start(dpool[hh:hh + 1, :], avs[64:65, :])
                        avs2.append(avs)
                    recp = nsb.tile([2, 512], f32r, name="recp", tag="recp")
                    nc.vector.reciprocal(recp[:], dpool[:])
                    deferred.append(mk_norm(p, q5, 0, avs2[0], recp))
                    deferred.append(mk_norm(p, q5, 1, avs2[1], recp))
                    if p == 3:
                        for tq in range(4):
                            deferred.append(mk_proj(q5, tq, wide=(q5 == 3)))
            # tail: drain the remaining deferred ops (pair 3 block 3)
            while deferred:
                deferred.popleft()()

    nc.finalize()
    return nc


def _host_inputs(x, key_padding_mask, w_qkv, b_qkv, w_out):
    """Per-core input dicts."""
    import ml_dtypes

    f32 = np.float32
    bf16 = ml_dtypes.bfloat16
    # masks (shared across cores): m01 = 1 where kept, 0 where col > row + L
    j = np.arange(128)[:, None]
    q = np.arange(512)[None, :]
    m01 = np.zeros((128, 5, 512), f32)
    for off in range(5):
        m01[:, off, :] = (128 * off + j <= q + L).astype(f32)
    m01 = m01.astype(bf16)
    vones = np.ones((128, NKT * HPG), bf16)
    selbc = np.zeros((2, 2 * HD), f32)
    selbc[0, 0:HD] = 1.0
    selbc[1, HD:2 * HD] = 1.0

    in_maps = []
    for c in range(NCORES):
        b, g = divmod(c, 2)
        # channel rows for this group's Q/K (pairs of heads -> 128 rows each)
        qrows = np.concatenate(
            [w_qkv[64 * (8 * g + 2 * p):64 * (8 * g + 2 * p) + 128] for p in range(NPAIR)])
        krows = np.concatenate(
            [w_qkv[D + 64 * (8 * g + 2 * p):D + 64 * (8 * g + 2 * p) + 128] for p in range(NPAIR)])
        vrows = w_qkv[2 * D + 512 * g:2 * D + 512 * g + 512]
        qkT = np.ascontiguousarray(
            np.concatenate([qrows, krows], 0).T)              # [1024, 1024]
        wqk = np.ascontiguousarray(
            qkT.reshape(FCH, 128, 8, 128).transpose(2, 0, 1, 3)).astype(bf16)
        wv = np.ascontiguousarray(vrows.T).reshape(FCH, 128, 512).astype(bf16)
        xT = np.ascontiguousarray(
            x[b].T.reshape(FCH, 128, TQ5, 512).transpose(0, 2, 1, 3)).astype(bf16)
        bq = np.stack(
            [b_qkv[64 * (8 * g + 2 * p):64 * (8 * g + 2 * p) + 128] for p in range(NPAIR)], 1)
        bk = np.stack(
            [b_qkv[D + 64 * (8 * g + 2 * p):D + 64 * (8 * g + 2 * p) + 128] for p in range(NPAIR)], 1)
        woutT = np.ascontiguousarray(
            w_out.T[512 * g:512 * g + 512]).reshape(NPAIR, 128, D).astype(bf16)
        kpb = np.ascontiguousarray(
            (NEG * key_padding_mask[b].astype(f32)).reshape(NKT, 128).T)
        in_maps.append({
            "xT": xT, "wqk": wqk, "wv": wv, "woutT": woutT,
            "bq": bq.astype(f32), "bk": bk.astype(f32), "kpb": kpb.astype(f32),
            "m01": m01, "vones": vones, "selbc": selbc,
        })
    return in_maps


def kernel(x, key_padding_mask, w_qkv, b_qkv, w_out, b_out):
    from concourse.bass_utils import run_bass_kernel_spmd

    x = np.asarray(x, np.float32)
    key_padding_mask = np.asarray(key_padding_mask)
    w_qkv = np.asarray(w_qkv, np.float32)
    b_qkv = np.asarray(b_qkv, np.float32)
    w_out = np.asarray(w_out, np.float32)
    b_out = np.asarray(b_out, np.float32)

    if "nc" not in _BUILT:
        _BUILT["nc"] = _build_nc()
    nc = _BUILT["nc"]

    in_maps = _host_inputs(x, key_padding_mask, w_qkv, b_qkv, w_out)
    res = run_bass_kernel_spmd(nc, in_maps, core_ids=list(range(NCORES)))
    out = np.empty((B, T, D), np.float32)
    for b in range(B):
        out[b] = res.results[2 * b]["out_part"] + res.results[2 * b + 1]["out_part"]
    # host-folded biases: b_out plus the V-bias pushed through the projection
    bv = b_qkv[2 * D:3 * D]
    out += (b_out + bv @ w_out.T)[None, None, :].astype(np.float32)
    return out
